# revision 10
# baseline (speedup 1.0000x reference)
"""Causal self-attention (B=4, S=2048, D=1024, single head) on 8 TRN2 cores.

Sharding: core c -> batch b = c//2, query-parity h = c%2. Each core computes
full K/V projections for its batch and attention for its 8 query tiles
(q-tiles 2s+h, s=0..7, 128 rows each). Every core runs an IDENTICAL program
(SPMD): slot s always processes E_s = 2(s+1) key tiles; a host-built additive
mask zeroes out the columns past the true causal extent, which differs only
by parity and therefore lives in the data, not the program.

Device layouts (per core):
  xt  [d=1024, s=2048]   x[b]^T                     (K/V projection operand)
  xq  [d=1024, q=1024]   x[b]^T gathered to own q-rows, slot-indexed
  w*t [d=1024, e=1024]   W^T for q/k/v               (shared across cores)
  kt  SBUF [128, 8*2048] K^T, e-group c at cols [c*2048:+2048]
  v   SBUF [128, 16*1024] V, s-tile g at cols [g*1024:+1024]
  qts DRAM [1024, 1024]  Q^T slot-indexed scratch (spilled during proj)
  out DRAM [1024, 1024]  slot-indexed rows; host scatters to [B,S,D]

All matmuls run as float32r (full fp32 data, 1 cycle/row on the PE when the
moving free dim is >= 256).
"""
import os
import sys

import numpy as np

for _p in ("/opt/trn_rl_repo", "/root/.axon_site/_ro/trn_rl_repo"):
    if os.path.isdir(_p) and _p not in sys.path:
        sys.path.insert(0, _p)

import concourse.bass as bass
import concourse.mybir as mybir
import concourse.tile as tile
from concourse.bass_utils import run_bass_kernel_spmd

B, S, D = 4, 2048, 1024
P = 128
SCALE = 1.0 / float(np.sqrt(D))
F32 = mybir.dt.float32
F32R = mybir.dt.float32r
NCORES = 8


def _legalize_single_wait(nc):
    """Walrus in this image encodes at most one sync wait per instruction.
    Split each multi-wait instruction into (n-1) prepended same-engine
    NoOps carrying one wait each (identical blocking semantics on an
    in-order engine)."""
    for fn in nc.m.functions:
        for block in fn.blocks:
            out = []
            for inst in block.instructions:
                si = inst.sync_info
                if si is not None and len(si.on_wait) > 1:
                    waits = list(si.on_wait)
                    for w in waits[:-1]:
                        out.append(mybir.InstNoOp(
                            name=nc.get_next_instruction_name(),
                            engine=inst.engine,
                            sync_info=mybir.SyncInfo(on_wait=[w],
                                                     on_update=[]),
                            bass_nofuse=True,
                            text_hint="waitsplit",
                        ))
                    inst.sync_info = mybir.SyncInfo(
                        on_wait=[waits[-1]], on_update=list(si.on_update))
                out.append(inst)
            try:
                block.instructions[:] = out
            except TypeError:
                block.instructions = out


def _build_program():
    nc = bass.Bass("TRN2", target_bir_lowering=False, debug=False,
                   num_devices=NCORES)

    xt = nc.dram_tensor("xt", [D, S], F32R, kind="ExternalInput").ap()
    xq = nc.dram_tensor("xq", [D, 1024], F32R, kind="ExternalInput").ap()
    wqt = nc.dram_tensor("wqt", [D, D], F32R, kind="ExternalInput").ap()
    wkt = nc.dram_tensor("wkt", [D, D], F32R, kind="ExternalInput").ap()
    wvt = nc.dram_tensor("wvt", [D, D], F32R, kind="ExternalInput").ap()
    mask = nc.dram_tensor("mask", [P, 16 * P], F32, kind="ExternalInput").ap()
    ident = nc.dram_tensor("ident", [P, P], F32, kind="ExternalInput").ap()
    out = nc.dram_tensor("out", [1024, D], F32, kind="ExternalOutput").ap()

    with tile.TileContext(nc) as tc:
        from contextlib import ExitStack

        # ---- persistent pools (both phases) ----
        persist = ExitStack()
        kt_pool = persist.enter_context(tc.tile_pool(name="ktp", bufs=1))
        v_pool = persist.enter_context(tc.tile_pool(name="vp", bufs=1))
        const_pool = persist.enter_context(tc.tile_pool(name="cst", bufs=1))
        dram_pool = persist.enter_context(
            tc.tile_pool(name="dscratch", bufs=1, space="DRAM"))

        kt = kt_pool.tile([P, 8 * S], F32R)      # K^T
        vv = v_pool.tile([P, 16 * D], F32R)      # V
        mk = const_pool.tile([P, 16 * P], F32)  # slot masks
        idn = const_pool.tile([P, P], F32)      # identity for PE transpose
        qts = dram_pool.tile([D, 1024], F32R)    # Q^T spill

        nc.sync.dma_start(out=mk[:], in_=mask)
        nc.sync.dma_start(out=idn[:], in_=ident)

        xt_v = xt.rearrange("(g p) s -> p g s", p=P)    # [128, 8, 2048]
        xq_v = xq.rearrange("(g p) q -> p g q", p=P)    # [128, 8, 1024]
        w_vs = {"q": wqt.rearrange("(g p) e -> p g e", p=P),
                "k": wkt.rearrange("(g p) e -> p g e", p=P),
                "v": wvt.rearrange("(g p) e -> p g e", p=P)}
        qts_v = qts[:].rearrange("(g p) q -> p g q", p=P)

        # ================= phase 1: projections =================
        with ExitStack() as ph1:
            xh_pool = ph1.enter_context(tc.tile_pool(name="xh", bufs=1))
            xqh_pool = ph1.enter_context(tc.tile_pool(name="xqh", bufs=1))
            w_pool = ph1.enter_context(tc.tile_pool(name="wsl", bufs=3))
            vt_pool = ph1.enter_context(tc.tile_pool(name="vtt", bufs=2))
            qs_pool = ph1.enter_context(tc.tile_pool(name="qst", bufs=2))
            ps_pool = ph1.enter_context(
                tc.tile_pool(name="psA", bufs=4, space="PSUM"))
            pt_pool = ph1.enter_context(
                tc.tile_pool(name="psT", bufs=2, space="PSUM"))

            for hp in range(2):  # s-halves of 1024
                xh = xh_pool.tile([P, 8 * 1024], F32R, tag="xh")
                nc.sync.dma_start(
                    out=xh[:].rearrange("p (g s) -> p g s", g=8),
                    in_=xt_v[:, :, hp * 1024:(hp + 1) * 1024])
                xqh = xqh_pool.tile([P, 8 * 512], F32R, tag="xqh")
                nc.sync.dma_start(
                    out=xqh[:].rearrange("p (g s) -> p g s", g=8),
                    in_=xq_v[:, :, hp * 512:(hp + 1) * 512])

                for c in range(8):  # e-tile of 128
                    wsl = {}
                    for pj in ("k", "v", "q"):
                        wt_ = w_pool.tile([P, 8 * P], F32R, tag="wsl",
                                          name=f"w{pj}{hp}{c}")
                        nc.sync.dma_start(
                            out=wt_[:].rearrange("p (g e) -> p g e", g=8),
                            in_=w_vs[pj][:, :, c * P:(c + 1) * P])
                        wsl[pj] = wt_

                    for j in range(2):  # 512-col s-chunk within half
                        # ---- K^T chunk [e-tile c, 512 s-cols] ----
                        pk = ps_pool.tile([P, 512], F32, tag="ps")
                        for g in range(8):
                            nc.tensor.matmul(
                                pk[:],
                                wsl["k"][:, g * P:(g + 1) * P],
                                xh[:, g * 1024 + j * 512:g * 1024 + (j + 1) * 512],
                                start=(g == 0), stop=(g == 7))
                        nc.scalar.copy(
                            kt[:, c * S + hp * 1024 + j * 512:
                               c * S + hp * 1024 + (j + 1) * 512], pk[:])

                        # ---- V^T chunk -> transpose -> V ----
                        pv = ps_pool.tile([P, 512], F32, tag="ps")
                        for g in range(8):
                            nc.tensor.matmul(
                                pv[:],
                                wsl["v"][:, g * P:(g + 1) * P],
                                xh[:, g * 1024 + j * 512:g * 1024 + (j + 1) * 512],
                                start=(g == 0), stop=(g == 7))
                        vt = vt_pool.tile([P, 512], F32, tag="vt")
                        nc.vector.tensor_copy(vt[:], pv[:])
                        pt = pt_pool.tile([P, 512], F32, tag="pt")
                        for t4 in range(4):
                            nc.tensor.transpose(
                                pt[:, t4 * P:(t4 + 1) * P],
                                vt[:, t4 * P:(t4 + 1) * P], idn[:])
                        # s-tile index g2 = hp*8 + j*4 + t4 -> cols g2*1024 + c*128
                        nc.vector.tensor_copy(
                            vv[:].rearrange("p (t e) -> p t e", t=16)
                            [:, hp * 8 + j * 4:hp * 8 + j * 4 + 4,
                             c * P:(c + 1) * P],
                            pt[:].rearrange("p (t e) -> p t e", t=4))

                    # ---- Q^T: two 256-col chunks (slots 4hp+2j, +1) ----
                    for j in range(2):
                        pq = ps_pool.tile([P, 512], F32, tag="ps")
                        for g in range(8):
                            nc.tensor.matmul(
                                pq[:, :256],
                                wsl["q"][:, g * P:(g + 1) * P],
                                xqh[:, g * 512 + j * 256:g * 512 + (j + 1) * 256],
                                start=(g == 0), stop=(g == 7))
                        qst = qs_pool.tile([P, 256], F32R, tag="qst")
                        nc.vector.tensor_copy(qst[:], pq[:, :256])
                        sA = 4 * hp + 2 * j
                        nc.sync.dma_start(
                            out=qts[c * P:(c + 1) * P, sA * P:sA * P + 256],
                            in_=qst[:])

        # ================= phase 2: attention =================
        with ExitStack() as ph2:
            qt_pool = ph2.enter_context(tc.tile_pool(name="qtl", bufs=2))
            w_sb_pool = ph2.enter_context(tc.tile_pool(name="wsb", bufs=2))
            wt_sb_pool = ph2.enter_context(tc.tile_pool(name="wtsb", bufs=2))
            o_pool = ph2.enter_context(tc.tile_pool(name="osb", bufs=2))
            st_pool = ph2.enter_context(tc.tile_pool(name="stat", bufs=8))
            psc_pool = ph2.enter_context(
                tc.tile_pool(name="psS", bufs=1, space="PSUM"))
            pso_pool = ph2.enter_context(
                tc.tile_pool(name="psO", bufs=1, space="PSUM"))
            pst_pool = ph2.enter_context(
                tc.tile_pool(name="psW", bufs=2, space="PSUM"))

            for s in range(8):
                E = 2 * (s + 1)          # k-tiles of 128
                L = E * P                # k-cols: 256..2048
                qt = qt_pool.tile([P, 8 * P], F32R, tag="qt")
                nc.sync.dma_start(
                    out=qt[:].rearrange("p (g q) -> p g q", g=8),
                    in_=qts_v[:, :, s * P:(s + 1) * P])

                sc = psc_pool.tile([P, 2048], F32, tag="sc")
                nch = (L + 511) // 512
                for kch in range(nch):
                    w = min(512, L - kch * 512)
                    for g in range(8):
                        nc.tensor.matmul(
                            sc[:, kch * 512:kch * 512 + w],
                            qt[:, g * P:(g + 1) * P],
                            kt[:, g * S + kch * 512:g * S + kch * 512 + w],
                            start=(g == 0), stop=(g == 7))

                # mask the trailing 256 cols (past-diagonal part)
                nc.vector.tensor_add(
                    sc[:, L - 256:L], sc[:, L - 256:L],
                    mk[:, s * 256:(s + 1) * 256])

                m = st_pool.tile([P, 1], F32, tag="st")
                nc.vector.reduce_max(m[:], sc[:, :L],
                                     axis=mybir.AxisListType.X)
                bias = st_pool.tile([P, 1], F32, tag="st")
                nc.scalar.mul(bias[:], m[:], -SCALE)

                w_sb = w_sb_pool.tile([P, 2048], F32R, tag="wsb")
                for kch in range(nch):
                    w = min(512, L - kch * 512)
                    nc.scalar.activation(
                        w_sb[:, kch * 512:kch * 512 + w],
                        sc[:, kch * 512:kch * 512 + w],
                        mybir.ActivationFunctionType.Exp,
                        bias=bias[:], scale=SCALE)

                ell = st_pool.tile([P, 1], F32, tag="st")
                nc.vector.reduce_sum(ell[:], w_sb[:, :L].bitcast(F32),
                                     axis=mybir.AxisListType.X)
                rinv = st_pool.tile([P, 1], F32, tag="st")
                nc.vector.reciprocal(rinv[:], ell[:])

                # transpose W (pack 4 tiles per PSUM bank)
                wt_sb = wt_sb_pool.tile([P, 2048], F32R, tag="wtsb")
                for bk in range((E + 3) // 4):
                    ntb = min(4, E - 4 * bk)
                    ptw = pst_pool.tile([P, 512], F32, tag="ptw")
                    for t4 in range(ntb):
                        ki = 4 * bk + t4
                        nc.tensor.transpose(
                            ptw[:, t4 * P:(t4 + 1) * P],
                            w_sb[:, ki * P:(ki + 1) * P].bitcast(F32), idn[:])
                    nc.vector.tensor_copy(
                        wt_sb[:, 4 * bk * P:4 * bk * P + ntb * P],
                        ptw[:, :ntb * P])

                # PV
                po = pso_pool.tile([P, 1024], F32, tag="po")
                for ki in range(E):
                    for eh in range(2):
                        nc.tensor.matmul(
                            po[:, eh * 512:(eh + 1) * 512],
                            wt_sb[:, ki * P:(ki + 1) * P],
                            vv[:, ki * D + eh * 512:ki * D + (eh + 1) * 512],
                            start=(ki == 0), stop=(ki == E - 1))

                o_sb = o_pool.tile([P, 1024], F32, tag="osb")
                nc.scalar.mul(o_sb[:], po[:], rinv[:])
                nc.sync.dma_start(out=out[s * P:(s + 1) * P, :], in_=o_sb[:])

        persist.close()

    _legalize_single_wait(nc)
    return nc


_NC = None


def _get_program():
    global _NC
    if _NC is None:
        _NC = _build_program()
    return _NC


def _make_mask(h):
    i = np.arange(P)[:, None]
    j2 = np.arange(256)[None, :]
    blk = np.where(j2 <= h * P + i, 0.0, -1e30).astype(np.float32)
    return np.tile(blk, (1, 8)).copy()


def _round_f32r(a):
    """Round fp32 to fp32r (11-bit mantissa, low 12 bits zero), RNE —
    matches walrus fp32_to_fp32r so DMA'd bytes are already rounded."""
    u = np.ascontiguousarray(a, dtype=np.float32).view(np.uint32)
    low = u & np.uint32(0xFFF)
    base = u & np.uint32(0xFFFFF000)
    rup = (low > 0x800) | ((low == 0x800) & (((u >> np.uint32(12)) & np.uint32(1)) == 1))
    out = base + (rup.astype(np.uint32) << np.uint32(12))
    return out.view(np.float32)


def _make_in_maps(x, Wq, Wk, Wv):
    x = _round_f32r(np.asarray(x, dtype=np.float32))
    wqt = _round_f32r(np.ascontiguousarray(np.asarray(Wq, dtype=np.float32).T))
    wkt = _round_f32r(np.ascontiguousarray(np.asarray(Wk, dtype=np.float32).T))
    wvt = _round_f32r(np.ascontiguousarray(np.asarray(Wv, dtype=np.float32).T))
    ident = np.eye(P, dtype=np.float32)
    masks = [_make_mask(0), _make_mask(1)]

    in_maps = []
    for c in range(NCORES):
        b, h = c // 2, c % 2
        xt = np.ascontiguousarray(x[b].T)
        own = np.concatenate([np.arange((2 * s + h) * P, (2 * s + h + 1) * P)
                              for s in range(8)])
        xq = np.ascontiguousarray(xt[:, own])
        in_maps.append({"xt": xt, "xq": xq, "wqt": wqt, "wkt": wkt,
                        "wvt": wvt, "mask": masks[h], "ident": ident})
    return in_maps


def kernel(x, Wq, Wk, Wv, _trace=False):
    in_maps = _make_in_maps(x, Wq, Wk, Wv)
    nc = _get_program()
    res = run_bass_kernel_spmd(nc, in_maps, list(range(NCORES)),
                               trace=_trace)

    out = np.empty((B, S, D), dtype=np.float32)
    for c in range(NCORES):
        b, h = c // 2, c % 2
        o = res.results[c]["out"]
        for s in range(8):
            out[b, (2 * s + h) * P:(2 * s + h + 1) * P, :] = \
                o[s * P:(s + 1) * P, :]
    if _trace:
        return out, res
    return out


if __name__ == "__main__":
    rng = np.random.default_rng(0)
    xs = rng.standard_normal((B, S, D), dtype=np.float32)
    ws = [rng.standard_normal((D, D), dtype=np.float32) * SCALE
          for _ in range(3)]
    o = kernel(xs, *ws)
    print("kernel ran, out shape", o.shape, "finite:", np.isfinite(o).all())
